# revision 53
# baseline (speedup 1.0000x reference)
"""DenseGCN Trainium2 kernel (8-core SPMD), v7.

Strategy (1D node partitioning, edge-cut by dst):
- Nodes range-sharded; edges on the dst-owning core, split into ev/od
  streams by src parity so the int16 SWDGE gather index src>>1 covers
  65536 nodes via a [N/2, 2H] bf16 pair-row table (256B rows).
- Edges bucketed by 128-node dst group AND SORTED BY SRC within each
  (group, stream) run, so each gather call's 256B HBM reads are in
  ascending address order (HBM row/bank friendly).
- The per-tile dst one-hot scatter matrix cm is precomputed on host in
  FP8-E4M3 (halves its HBM traffic) and streamed as the MOVING matmul
  operand.
- Scatter matmul is FLIPPED vs v6: lhsT = gathered tile [128e, 64f]
  (stationary, 64-col LDWEIGHTS), rhs = cm [128e, 128d] -> out is the
  TRANSPOSED aggregate aggT [64f, 128d] in PSUM. This removes the
  per-group PE transpose + a PSUM copy, and the conv matmul consumes
  aggT directly as lhsT.
- Three PSUM accumulators rotate across a group's tiles so consecutive
  matmuls never target the same PSUM bank (hides the ~175ns drain).
- Conv LN+residual is deferred and batched per 16-group chunk (DVE op
  count cut ~2x vs per-group LN), overlapping the remaining groups.
- Gathers run on 4 SWDGE queues round-robin (1024 idx per call) with 8
  rotating destination buffers.
- fc_first / fc_final run stage-major as in v6.
"""

import math

import numpy as np

import concourse.bacc as bacc
import concourse.bass as bass
import concourse.mybir as mybir
import concourse.tile as tile
from concourse import library_config
from concourse.bass_utils import run_bass_kernel_spmd

F32 = mybir.dt.float32
BF16 = mybir.dt.bfloat16
FP8 = mybir.dt.float8e4
I16 = mybir.dt.int16
I32 = mybir.dt.int32
AF = mybir.ActivationFunctionType
ALU = mybir.AluOpType
AX = mybir.AxisListType

NC_CORES = 8
F_IN = 128
H = 64
OUT = 32
L = 3
EPS = 1e-5
GROUP = 128
P = 128
NQ = 4            # SWDGE queues
TPC = 8           # tiles per gather call (1024 idx = ring capacity)
CMB = 16          # tiles per cm load (512 KB bf16: amortize HWDGE fixed cost)
LNCHUNK = 8       # groups per batched-LN chunk
NACC = 2          # rotating PSUM accumulators per group (PSUM bank budget)


def _wrap_idx16(idx, E_s):
    """[E_s] int -> [128, E_s/16] int16, 16-partition wrap replicated 8x."""
    assert E_s % 16 == 0
    w16 = idx.reshape(E_s // 16, 16).T.astype(np.int16)  # [16, E_s/16]
    return np.ascontiguousarray(np.tile(w16, (8, 1)))  # [128, E_s/16]


def prep_inputs(x, edge_weight, src, dst, n_nodes, npc):
    """Host-side shard + edge bucketing + fp8 cm precompute.

    Edges go to the core owning dst, split into ev (src even) / od (src
    odd) streams, bucketed by 128-node dst group, sorted by src within
    each (group, stream) run, padded to whole 128-edge tiles; all cores
    padded to the same K_ev / K_od tiles per group.
    """
    import ml_dtypes

    ew = edge_weight.reshape(-1).astype(np.float32)
    src = src.astype(np.int64)
    dst = dst.astype(np.int64)
    ngroups = npc // GROUP

    per_core = []
    kmax = [1, 1]
    for c in range(NC_CORES):
        m = (dst // npc) == c
        s_c, d_c, w_c = src[m], dst[m], ew[m]
        halves = []
        for h in range(2):
            hm = (s_c % 2) == h
            s_h, d_h, w_h = s_c[hm], d_c[hm], w_c[hm]
            g = (d_h % npc) // GROUP
            # sort by (group, src): ascending gather addresses per run
            order = np.lexsort((s_h, g))
            s_h, d_h, w_h, g = s_h[order], d_h[order], w_h[order], g[order]
            cnt = np.bincount(g, minlength=ngroups)
            kmax[h] = max(kmax[h], math.ceil(int(cnt.max()) / P))
            halves.append((s_h, d_h, w_h, g, cnt))
        per_core.append(halves)

    K_ev, K_od = kmax

    maps = []
    for c in range(NC_CORES):
        out = {
            "x": np.ascontiguousarray(x[c * npc : (c + 1) * npc]).astype(
                np.float32
            )
        }
        for h, K in ((0, K_ev), (1, K_od)):
            s_h, d_h, w_h, g, cnt = per_core[c][h]
            E_s = ngroups * K * P
            T = E_s // P
            starts = np.zeros(ngroups, dtype=np.int64)
            starts[1:] = np.cumsum(cnt)[:-1]
            within = np.arange(len(g)) - starts[g]
            slot = g * (K * P) + within          # edge slot e = t*128 + p
            idx = np.zeros(E_s, dtype=np.int64)
            idx[slot] = s_h >> 1                 # pair-row index
            # cm wide layout: cmw[p, t*128 + dstloc] = ew
            p_arr = slot % P
            t_arr = slot // P
            dl = (d_h % GROUP).astype(np.int64)
            cmw = np.zeros((P, T * P), dtype=np.float32)
            cmw[p_arr, t_arr * P + dl] = w_h
            sfx = "od" if h else "ev"
            out[f"eidx_{sfx}"] = _wrap_idx16(idx, E_s)
            out[f"cm_{sfx}"] = np.ascontiguousarray(
                cmw.astype(ml_dtypes.bfloat16)
            )
        maps.append(out)
    return maps, (K_ev, K_od)


def build_nc(n_nodes, npc, K_lh, ln_identity):
    """Build the SPMD Bass program (same program all 8 cores)."""
    K_ev, K_od = K_lh
    ngroups = npc // GROUP
    ntile_node = npc // P
    T_ev = ngroups * K_ev
    T_od = ngroups * K_od
    assert TPC * P <= 1024

    nc = bacc.Bacc(None, target_bir_lowering=False, num_swdge_queues=NQ)

    # ---- I/O ----
    x_d = nc.declare_dram_parameter("x", [npc, F_IN], F32, isOutput=False)
    eidx_d, cm_d = {}, {}
    for sfx, T in (("ev", T_ev), ("od", T_od)):
        eidx_d[sfx] = nc.declare_dram_parameter(
            f"eidx_{sfx}", [P, T * P // 16], I16, isOutput=False
        )
        cm_d[sfx] = nc.declare_dram_parameter(
            f"cm_{sfx}", [P, T * P], BF16, isOutput=False
        )
    w1_d = nc.declare_dram_parameter("w1", [F_IN, H], F32, isOutput=False)
    b1_d = nc.declare_dram_parameter("b1r", [P, H], F32, isOutput=False)
    cw_d = [
        nc.declare_dram_parameter(f"cw{i}", [H, H], F32, isOutput=False)
        for i in range(L)
    ]
    cb_d = [
        nc.declare_dram_parameter(f"cb{i}r", [P, H], F32, isOutput=False)
        for i in range(L)
    ]
    w3_d = nc.declare_dram_parameter("w3", [H, H], F32, isOutput=False)
    b3_d = nc.declare_dram_parameter("b3r", [P, H], F32, isOutput=False)
    w4_d = nc.declare_dram_parameter("w4", [H, OUT], F32, isOutput=False)
    b4_d = nc.declare_dram_parameter("b4r", [P, OUT], F32, isOutput=False)
    ident_d = nc.declare_dram_parameter("ident", [P, P], F32, isOutput=False)
    ident2_d = nc.declare_dram_parameter("ident2", [P, H], F32, isOutput=False)
    ln_d = {}
    if not ln_identity:
        ln_d["ln1g"] = nc.declare_dram_parameter("ln1g", [P, F_IN], F32, False)
        ln_d["ln1b"] = nc.declare_dram_parameter("ln1b", [P, F_IN], F32, False)
        ln_d["lng"] = nc.declare_dram_parameter("lng", [P, H], F32, False)
        ln_d["lnb"] = nc.declare_dram_parameter("lnb", [P, H], F32, False)
        ln_d["ln2g"] = nc.declare_dram_parameter("ln2g", [P, H], F32, False)
        ln_d["ln2b"] = nc.declare_dram_parameter("ln2b", [P, H], F32, False)
    out_d = nc.declare_dram_parameter("out", [npc, OUT], F32, isOutput=True)

    # ---- internal DRAM ----
    warm_out = nc.dram_tensor(
        "warm_out", [NC_CORES * P, H], BF16, addr_space="Shared"
    )
    h_bounce = nc.dram_tensor("h_bounce", [npc, H], BF16)
    # bf16 pair-row gather table: row i = nodes (2i, 2i+1)
    h_pair = nc.dram_tensor(
        "h_pair", [n_nodes // 2, 2 * H], BF16, addr_space="Shared"
    )

    groups_all = [list(range(NC_CORES))]

    with tile.TileContext(nc) as tc:
        with (
            tc.tile_pool(name="const", bufs=1) as cpool,
            tc.tile_pool(name="gpool", bufs=4) as gpool,
            tc.tile_pool(name="cmp", bufs=2) as cmpool,
            tc.tile_pool(name="work", bufs=4) as wpool,
            tc.tile_pool(name="wide", bufs=2) as widepool,
            tc.tile_pool(name="stat", bufs=8) as spool,
            tc.tile_pool(name="msgp", bufs=6) as mpool,
            tc.tile_pool(name="psA", bufs=2, space="PSUM") as psA,
            tc.tile_pool(name="psB", bufs=2, space="PSUM") as psB,
            tc.tile_pool(name="psC", bufs=2, space="PSUM") as psC,
            tc.tile_pool(name="psD", bufs=2, space="PSUM") as psD,
        ):
            # ---------- persistent constants ----------
            nc.gpsimd.load_library(library_config.mlp)
            ident = cpool.tile([P, P], F32)
            nc.sync.dma_start(out=ident[:], in_=ident_d[:, :])
            ident2_bf = cpool.tile([P, H], BF16, tag="ident2bf")
            ident2_f = wpool.tile([P, H], F32, tag="ident2f")
            nc.sync.dma_start(out=ident2_f[:], in_=ident2_d[:, :])
            nc.vector.tensor_copy(ident2_bf[:], ident2_f[:])

            eidx_s = {}
            for sfx, T in (("ev", T_ev), ("od", T_od)):
                eidx_s[sfx] = cpool.tile(
                    [P, T * P // 16], I16, tag=f"ei{sfx}", name=f"eidx_s_{sfx}"
                )
                nc.sync.dma_start(out=eidx_s[sfx][:], in_=eidx_d[sfx][:, :])

            w1_s = cpool.tile([F_IN, H], F32)
            nc.sync.dma_start(out=w1_s[:], in_=w1_d[:, :])
            b1_s = cpool.tile([P, H], F32)
            nc.sync.dma_start(out=b1_s[:], in_=b1_d[:, :])
            cw_s, cb_s = [], []
            for i in range(L):
                w = cpool.tile([H, H], F32, tag=f"cw{i}")
                nc.sync.dma_start(out=w[:], in_=cw_d[i][:, :])
                cw_s.append(w)
                b = cpool.tile([P, H], F32, tag=f"cb{i}")
                nc.sync.dma_start(out=b[:], in_=cb_d[i][:, :])
                cb_s.append(b)
            w3_s = cpool.tile([H, H], F32, tag="w3")
            nc.sync.dma_start(out=w3_s[:], in_=w3_d[:, :])
            b3_s = cpool.tile([P, H], F32, tag="b3")
            nc.sync.dma_start(out=b3_s[:], in_=b3_d[:, :])
            w4_s = cpool.tile([H, OUT], F32, tag="w4")
            nc.sync.dma_start(out=w4_s[:], in_=w4_d[:, :])
            b4_s = cpool.tile([P, OUT], F32, tag="b4")
            nc.sync.dma_start(out=b4_s[:], in_=b4_d[:, :])
            ln_s = {}
            for k in ln_d:
                f = F_IN if k.startswith("ln1") else H
                t_ = cpool.tile([P, f], F32, tag=k)
                nc.sync.dma_start(out=t_[:], in_=ln_d[k][:, :])
                ln_s[k] = t_

            dma_engines = [nc.sync, nc.scalar]

            h_stage = cpool.tile([P, ntile_node * H], F32, tag="hstage")
            h_stage_bf = cpool.tile(
                [P, ntile_node * H], BF16, tag="hstagebf"
            )
            hb_stage = cpool.tile([P, ntile_node * H], F32, tag="hbstage")
            out_stage = cpool.tile([P, ntile_node * OUT], F32, tag="ostage")

            # ---------- helpers ----------
            def layer_norm(dst_ap, src_ap, f, gkey=None, bkey=None,
                           residual_ap=None):
                """dst = LN(src) along free axis of width f (DVE-only)."""
                parts = src_ap.shape[0]
                ssum = spool.tile([P, 1], F32, tag="lnsum")
                nc.vector.tensor_reduce(
                    out=ssum[:parts], in_=src_ap, axis=AX.X, op=ALU.add
                )
                xc = wpool.tile([P, f], F32, tag=f"lnxc{f}")
                nc.vector.scalar_tensor_tensor(
                    out=xc[:parts],
                    in0=src_ap,
                    scalar=float(f),
                    in1=ssum[:parts].to_broadcast([parts, f]),
                    op0=ALU.mult,
                    op1=ALU.subtract,
                )
                sq = wpool.tile([P, f], F32, tag=f"lnsq{f}")
                nc.vector.tensor_tensor(
                    out=sq[:parts], in0=xc[:parts], in1=xc[:parts], op=ALU.mult
                )
                vsum = spool.tile([P, 1], F32, tag="lnvar")
                nc.vector.tensor_reduce(
                    out=vsum[:parts], in_=sq[:parts], axis=AX.X, op=ALU.add
                )
                veps = spool.tile([P, 1], F32, tag="lnveps")
                nc.vector.tensor_scalar(
                    out=veps[:parts],
                    in0=vsum[:parts],
                    scalar1=1.0 / f,
                    scalar2=float(f) * float(f) * EPS,
                    op0=ALU.mult,
                    op1=ALU.add,
                )
                rstd = spool.tile([P, 1], F32, tag="lnrstd")
                ri = rstd[:parts].bitcast(I32)
                nc.vector.tensor_scalar(
                    out=ri,
                    in0=veps[:parts].bitcast(I32),
                    scalar1=1,
                    scalar2=None,
                    op0=ALU.logical_shift_right,
                )
                nc.vector.tensor_scalar(
                    out=ri,
                    in0=ri,
                    scalar1=-1,
                    scalar2=0x5F3759DF,
                    op0=ALU.mult,
                    op1=ALU.add,
                )
                ysq = spool.tile([P, 1], F32, tag="lnysq")
                half_t = spool.tile([P, 1], F32, tag="lnht")
                for _ in range(2):
                    nc.vector.tensor_tensor(
                        out=ysq[:parts], in0=rstd[:parts], in1=rstd[:parts],
                        op=ALU.mult,
                    )
                    nc.vector.scalar_tensor_tensor(
                        out=half_t[:parts],
                        in0=veps[:parts],
                        scalar=-0.5,
                        in1=ysq[:parts],
                        op0=ALU.mult,
                        op1=ALU.mult,
                    )
                    nc.vector.scalar_tensor_tensor(
                        out=rstd[:parts],
                        in0=half_t[:parts],
                        scalar=1.5,
                        in1=rstd[:parts],
                        op0=ALU.add,
                        op1=ALU.mult,
                    )
                rstd_b = rstd[:parts].to_broadcast([parts, f])
                if gkey is None:
                    if residual_ap is not None:
                        nrm = wpool.tile([P, f], F32, tag=f"lnnrm{f}")
                        nc.vector.tensor_tensor(
                            out=nrm[:parts], in0=xc[:parts], in1=rstd_b,
                            op=ALU.mult,
                        )
                        nc.vector.tensor_tensor(
                            out=dst_ap, in0=nrm[:parts], in1=residual_ap,
                            op=ALU.add,
                        )
                    else:
                        nc.vector.tensor_tensor(
                            out=dst_ap, in0=xc[:parts], in1=rstd_b,
                            op=ALU.mult,
                        )
                else:
                    nrm = wpool.tile([P, f], F32, tag=f"lnnrm{f}")
                    nc.vector.tensor_tensor(
                        out=nrm[:parts], in0=xc[:parts], in1=rstd_b,
                        op=ALU.mult,
                    )
                    tmp = wpool.tile([P, f], F32, tag=f"lnaf{f}")
                    nc.vector.tensor_tensor(
                        out=tmp[:parts],
                        in0=nrm[:parts],
                        in1=ln_s[gkey][:parts],
                        op=ALU.mult,
                    )
                    nc.vector.tensor_tensor(
                        out=dst_ap,
                        in0=tmp[:parts],
                        in1=ln_s[bkey][:parts],
                        op=ALU.add,
                    )
                    if residual_ap is not None:
                        nc.vector.tensor_tensor(
                            out=dst_ap,
                            in0=dst_ap,
                            in1=residual_ap,
                            op=ALU.add,
                        )

            def ln_batch(dst_stage, src_stage, f, nt):
                """Batched identity-LN over nt tiles of width f."""
                stats_s = spool.tile([P, nt], F32, tag=f"lbs{f}")
                stats_v = spool.tile([P, nt], F32, tag=f"lbv{f}")
                for t in range(nt):
                    ssl = slice(t * f, (t + 1) * f)
                    nc.vector.tensor_reduce(
                        out=stats_s[:, t : t + 1], in_=src_stage[:, ssl],
                        axis=AX.X, op=ALU.add,
                    )
                    nc.vector.scalar_tensor_tensor(
                        out=dst_stage[:, ssl],
                        in0=src_stage[:, ssl],
                        scalar=float(f),
                        in1=stats_s[:, t : t + 1].to_broadcast([P, f]),
                        op0=ALU.mult,
                        op1=ALU.subtract,
                    )
                    sq = wpool.tile([P, f], F32, tag=f"lnsq{f}")
                    nc.vector.tensor_tensor(
                        out=sq[:], in0=dst_stage[:, ssl],
                        in1=dst_stage[:, ssl], op=ALU.mult,
                    )
                    nc.vector.tensor_reduce(
                        out=stats_v[:, t : t + 1], in_=sq[:], axis=AX.X,
                        op=ALU.add,
                    )
                veps = spool.tile([P, nt], F32, tag=f"lbe{f}")
                nc.vector.tensor_scalar(
                    out=veps[:],
                    in0=stats_v[:],
                    scalar1=1.0 / f,
                    scalar2=float(f) * float(f) * EPS,
                    op0=ALU.mult,
                    op1=ALU.add,
                )
                rstd = spool.tile([P, nt], F32, tag=f"lbr{f}")
                ri = rstd[:].bitcast(I32)
                nc.vector.tensor_scalar(
                    out=ri, in0=veps[:].bitcast(I32), scalar1=1,
                    scalar2=None, op0=ALU.logical_shift_right,
                )
                nc.vector.tensor_scalar(
                    out=ri, in0=ri, scalar1=-1, scalar2=0x5F3759DF,
                    op0=ALU.mult, op1=ALU.add,
                )
                ysq = spool.tile([P, nt], F32, tag=f"lby{f}")
                half_t = spool.tile([P, nt], F32, tag=f"lbh{f}")
                for _ in range(2):
                    nc.vector.tensor_tensor(
                        out=ysq[:], in0=rstd[:], in1=rstd[:], op=ALU.mult
                    )
                    nc.vector.scalar_tensor_tensor(
                        out=half_t[:], in0=veps[:], scalar=-0.5,
                        in1=ysq[:], op0=ALU.mult, op1=ALU.mult,
                    )
                    nc.vector.scalar_tensor_tensor(
                        out=rstd[:], in0=half_t[:], scalar=1.5,
                        in1=rstd[:], op0=ALU.add, op1=ALU.mult,
                    )
                for t in range(nt):
                    ssl = slice(t * f, (t + 1) * f)
                    nc.vector.tensor_tensor(
                        out=dst_stage[:, ssl],
                        in0=dst_stage[:, ssl],
                        in1=rstd[:, t : t + 1].to_broadcast([P, f]),
                        op=ALU.mult,
                    )

            def ln_chunk_residual(h_ap_full, pre_stage, c0, nt, gkey, bkey):
                """h[:, chunk] = LN(pre[:, chunk]) (*g+b) + h[:, chunk].

                pre_stage holds the pre-LN conv outputs (modified in place
                to the centered-scaled value); h_ap_full supplies the
                residual (old h) and receives the result.
                """
                f = H
                stats_s = spool.tile([P, nt], F32, tag="lcs")
                stats_v = spool.tile([P, nt], F32, tag="lcv")
                for t in range(nt):
                    ssl = slice((c0 + t) * f, (c0 + t + 1) * f)
                    nc.vector.tensor_reduce(
                        out=stats_s[:, t : t + 1], in_=pre_stage[:, ssl],
                        axis=AX.X, op=ALU.add,
                    )
                    nc.vector.scalar_tensor_tensor(
                        out=pre_stage[:, ssl],
                        in0=pre_stage[:, ssl],
                        scalar=float(f),
                        in1=stats_s[:, t : t + 1].to_broadcast([P, f]),
                        op0=ALU.mult,
                        op1=ALU.subtract,
                    )
                    sq = wpool.tile([P, f], F32, tag="lcsq")
                    nc.vector.tensor_tensor(
                        out=sq[:], in0=pre_stage[:, ssl],
                        in1=pre_stage[:, ssl], op=ALU.mult,
                    )
                    nc.vector.tensor_reduce(
                        out=stats_v[:, t : t + 1], in_=sq[:], axis=AX.X,
                        op=ALU.add,
                    )
                veps = spool.tile([P, nt], F32, tag="lce")
                nc.vector.tensor_scalar(
                    out=veps[:],
                    in0=stats_v[:],
                    scalar1=1.0 / f,
                    scalar2=float(f) * float(f) * EPS,
                    op0=ALU.mult,
                    op1=ALU.add,
                )
                rstd = spool.tile([P, nt], F32, tag="lcr")
                ri = rstd[:].bitcast(I32)
                nc.vector.tensor_scalar(
                    out=ri, in0=veps[:].bitcast(I32), scalar1=1,
                    scalar2=None, op0=ALU.logical_shift_right,
                )
                nc.vector.tensor_scalar(
                    out=ri, in0=ri, scalar1=-1, scalar2=0x5F3759DF,
                    op0=ALU.mult, op1=ALU.add,
                )
                ysq = spool.tile([P, nt], F32, tag="lcy")
                half_t = spool.tile([P, nt], F32, tag="lch")
                for _ in range(2):
                    nc.vector.tensor_tensor(
                        out=ysq[:], in0=rstd[:], in1=rstd[:], op=ALU.mult
                    )
                    nc.vector.scalar_tensor_tensor(
                        out=half_t[:], in0=veps[:], scalar=-0.5,
                        in1=ysq[:], op0=ALU.mult, op1=ALU.mult,
                    )
                    nc.vector.scalar_tensor_tensor(
                        out=rstd[:], in0=half_t[:], scalar=1.5,
                        in1=rstd[:], op0=ALU.add, op1=ALU.mult,
                    )
                for t in range(nt):
                    ssl = slice((c0 + t) * f, (c0 + t + 1) * f)
                    nrm = wpool.tile([P, f], F32, tag="lcnrm")
                    nc.vector.tensor_tensor(
                        out=nrm[:],
                        in0=pre_stage[:, ssl],
                        in1=rstd[:, t : t + 1].to_broadcast([P, f]),
                        op=ALU.mult,
                    )
                    if gkey is not None:
                        nc.vector.tensor_tensor(
                            out=nrm[:], in0=nrm[:], in1=ln_s[gkey][:],
                            op=ALU.mult,
                        )
                        nc.vector.tensor_tensor(
                            out=nrm[:], in0=nrm[:], in1=ln_s[bkey][:],
                            op=ALU.add,
                        )
                    nc.vector.tensor_tensor(
                        out=h_ap_full[:, ssl], in0=nrm[:],
                        in1=h_ap_full[:, ssl], op=ALU.add,
                    )

            def elu_wide(dst_ap, src_ap, width, chunk=512):
                for o in range(0, width, chunk):
                    w = min(chunk, width - o)
                    s_ap = src_ap[:, o : o + w]
                    d_ap = dst_ap[:, o : o + w]
                    r1 = widepool.tile([P, chunk], F32, tag="elur1w")
                    nc.vector.tensor_scalar(
                        out=r1[:, :w],
                        in0=s_ap,
                        scalar1=0.0,
                        scalar2=1.0,
                        op0=ALU.max,
                        op1=ALU.subtract,
                    )
                    mn = widepool.tile([P, chunk], F32, tag="elumnw")
                    nc.vector.tensor_scalar(
                        out=mn[:, :w], in0=s_ap, scalar1=0.0, scalar2=None,
                        op0=ALU.min,
                    )
                    ex = widepool.tile([P, chunk], F32, tag="eluexw")
                    nc.scalar.activation(ex[:, :w], mn[:, :w], AF.Exp)
                    nc.vector.tensor_tensor(
                        out=d_ap, in0=r1[:, :w], in1=ex[:, :w], op=ALU.add
                    )

            # warmup collective
            nc.gpsimd.collective_compute(
                "AllGather",
                ALU.bypass,
                replica_groups=groups_all,
                ins=[h_bounce[0:P, :]],
                outs=[warm_out[:, :]],
            )

            # ---------- fc_first ----------
            if ln_identity:
                # one big x load + batched LN1 in place, then per-tile linear
                x_stage = cpool.tile(
                    [P, ntile_node * F_IN], F32, tag="xstage"
                )
                nc.sync.dma_start(
                    out=x_stage[:].rearrange("p (t f) -> p t f", f=F_IN),
                    in_=x_d[:, :].rearrange("(t p) f -> p t f", p=P),
                )
                ln_batch(x_stage, x_stage, F_IN, ntile_node)
                for t in range(ntile_node):
                    fsl = slice(t * F_IN, (t + 1) * F_IN)
                    xT_ps = psD.tile([P, P], F32, tag="trps")
                    nc.tensor.transpose(
                        out=xT_ps[:], in_=x_stage[:, fsl], identity=ident[:]
                    )
                    xT = wpool.tile([P, P], F32, tag="xT")
                    nc.scalar.copy(xT[:], xT_ps[:])
                    h_ps = psC.tile([P, H], F32, tag="linps")
                    nc.tensor.matmul(
                        out=h_ps[:], lhsT=xT[:], rhs=w1_s[:], start=True,
                        stop=True,
                    )
                    sl = slice(t * H, (t + 1) * H)
                    nc.vector.tensor_tensor(
                        out=hb_stage[:, sl], in0=h_ps[:], in1=b1_s[:],
                        op=ALU.add,
                    )
            else:
                for t in range(ntile_node):
                    xt = wpool.tile([P, F_IN], F32, tag="xt")
                    nc.sync.dma_start(
                        out=xt[:], in_=x_d[t * P : (t + 1) * P, :]
                    )
                    lnt = wpool.tile([P, F_IN], F32, tag="lnt")
                    layer_norm(lnt[:], xt[:], F_IN, "ln1g", "ln1b")
                    xT_ps = psD.tile([P, P], F32, tag="trps")
                    nc.tensor.transpose(
                        out=xT_ps[:], in_=lnt[:], identity=ident[:]
                    )
                    xT = wpool.tile([P, P], F32, tag="xT")
                    nc.scalar.copy(xT[:], xT_ps[:])
                    h_ps = psC.tile([P, H], F32, tag="linps")
                    nc.tensor.matmul(
                        out=h_ps[:], lhsT=xT[:], rhs=w1_s[:], start=True,
                        stop=True,
                    )
                    sl = slice(t * H, (t + 1) * H)
                    nc.vector.tensor_tensor(
                        out=hb_stage[:, sl], in0=h_ps[:], in1=b1_s[:],
                        op=ALU.add,
                    )
            elu_wide(hb_stage[:], hb_stage[:], ntile_node * H)
            if ln_identity:
                ln_batch(h_stage, hb_stage, H, ntile_node)
            else:
                for t in range(ntile_node):
                    sl = slice(t * H, (t + 1) * H)
                    layer_norm(h_stage[:, sl], hb_stage[:, sl], H,
                               "lng", "lnb")

            def bounce_chunk(c0, nt):
                """Cast + DMA groups [c0, c0+nt) of h_stage to h_bounce so
                only the AllGather itself remains at the layer boundary."""
                csl = slice(c0 * H, (c0 + nt) * H)
                nc.vector.tensor_copy(h_stage_bf[:, csl], h_stage[:, csl])
                hbv = h_bounce[c0 * P : (c0 + nt) * P, :].rearrange(
                    "(t p) f -> p t f", p=P
                )
                nc.sync.dma_start(
                    out=hbv,
                    in_=h_stage_bf[:, csl].rearrange("p (t f) -> p t f", f=H),
                )

            bounce_chunk(0, ntile_node)

            hb_v = h_bounce[:, :].rearrange("(t p) f -> p t f", p=P)

            lngk = None if ln_identity else "lng"
            lnbk = None if ln_identity else "lnb"

            # ---------- conv layers ----------
            for li in range(L):
                nc.gpsimd.collective_compute(
                    "AllGather",
                    ALU.bypass,
                    replica_groups=groups_all,
                    ins=[h_bounce[:, :]],
                    outs=[h_pair[:, :]],
                )
                gbufs = {"ev": None, "od": None}
                qn = [0]

                cms_cur = {}
                cms_nxt = {}
                cur_b = {"ev": 0, "od": 0}

                def load_cm_batch(sfx, b, T):
                    cb0 = b * CMB
                    n_c = min(CMB, T - cb0)
                    tl = cmpool.tile(
                        [P, CMB, P], BF16, tag=f"cms{sfx}", name=f"cms_{sfx}"
                    )
                    eng = dma_engines[b % len(dma_engines)]
                    eng.dma_start(
                        out=tl[:, :n_c, :],
                        in_=cm_d[sfx][
                            :, cb0 * P : (cb0 + n_c) * P
                        ].rearrange("p (t d) -> p t d", d=P),
                    )
                    return tl

                for sfx, T in (("ev", T_ev), ("od", T_od)):
                    cms_cur[sfx] = load_cm_batch(sfx, 0, T)
                    cms_nxt[sfx] = (
                        load_cm_batch(sfx, 1, T) if T > CMB else None
                    )
                    cur_b[sfx] = 0

                for g in range(ngroups):
                    accs = [
                        psA.tile([H, P], F32, tag=f"acc{a}", name=f"acc{a}")
                        for a in range(NACC)
                    ]
                    tiles = [("ev", 0, K_ev, T_ev, k) for k in range(K_ev)] + [
                        ("od", H, K_od, T_od, k) for k in range(K_od)
                    ]
                    ntl = len(tiles)
                    nchain = min(NACC, ntl)
                    for i, (sfx, half0, K, T, k) in enumerate(tiles):
                        t = g * K + k
                        tt = t % TPC
                        if tt == 0:
                            c0 = t
                            n_t = min(TPC, T - c0)
                            n_e = n_t * P
                            gbufs[sfx] = gpool.tile(
                                [P, n_t, 2 * H], BF16, tag=f"gbuf{sfx}",
                                name=f"gbuf_{sfx}"
                            )
                            nc.gpsimd.dma_gather(
                                out_ap=gbufs[sfx][:],
                                in_ap=h_pair[:, :],
                                idxs_ap=eidx_s[sfx][
                                    :,
                                    c0 * (P // 16) : c0 * (P // 16)
                                    + (n_e // 16),
                                ],
                                num_idxs=n_e,
                                num_idxs_reg=n_e,
                                elem_size=2 * H,
                                queue_num=qn[0] % NQ,
                                single_packet=False,
                            )
                            qn[0] += 1
                        if t // CMB > cur_b[sfx]:
                            cur_b[sfx] += 1
                            cms_cur[sfx] = cms_nxt[sfx]
                            nb = cur_b[sfx] + 1
                            cms_nxt[sfx] = (
                                load_cm_batch(sfx, nb, T)
                                if nb * CMB < T
                                else None
                            )
                        ch = i % nchain
                        ct = t % CMB
                        # flipped matmul: stationary = gathered [128e, 64f],
                        # moving = one-hot [128e, 128d] -> aggT [64f, 128d]
                        nc.tensor.matmul(
                            out=accs[ch][:],
                            lhsT=gbufs[sfx][:, tt, half0 : half0 + H],
                            rhs=cms_cur[sfx][:, ct, :],
                            start=(i < nchain),
                            stop=(i + nchain >= ntl),
                        )
                    # --- group epilogue: sum accumulators, conv, bias ---
                    aggT_s = wpool.tile([H, P], F32, tag="aggTs")
                    nc.scalar.copy(aggT_s[:], accs[0][:])
                    for a in range(1, nchain):
                        nc.vector.tensor_tensor(
                            out=aggT_s[:], in0=aggT_s[:], in1=accs[a][:],
                            op=ALU.add,
                        )
                    lin_ps = psC.tile([P, H], F32, tag="linps")
                    nc.tensor.matmul(
                        out=lin_ps[:],
                        lhsT=aggT_s[:],
                        rhs=cw_s[li][:],
                        start=True,
                        stop=True,
                    )
                    gsl = slice(g * H, (g + 1) * H)
                    nc.vector.tensor_tensor(
                        out=hb_stage[:, gsl], in0=lin_ps[:], in1=cb_s[li][:],
                        op=ALU.add,
                    )
                    # batched deferred LN + residual per LNCHUNK groups
                    if (g + 1) % LNCHUNK == 0:
                        ln_chunk_residual(
                            h_stage, hb_stage, g + 1 - LNCHUNK, LNCHUNK,
                            lngk, lnbk,
                        )
                        if li < L - 1:
                            bounce_chunk(g + 1 - LNCHUNK, LNCHUNK)

            # ---------- fc_final (stage-major) ----------
            if ln_identity:
                ln_batch(hb_stage, h_stage, H, ntile_node)
            else:
                for t in range(ntile_node):
                    sl = slice(t * H, (t + 1) * H)
                    layer_norm(hb_stage[:, sl], h_stage[:, sl], H,
                               "ln2g", "ln2b")
            for t in range(ntile_node):
                sl = slice(t * H, (t + 1) * H)
                tr_ps = psD.tile([P, P], F32, tag="trps")
                nc.tensor.transpose(
                    out=tr_ps[:H, :], in_=hb_stage[:, sl], identity=ident[:]
                )
                lnhT = wpool.tile([H, P], F32, tag="aggTs")
                nc.scalar.copy(lnhT[:], tr_ps[:H, :])
                z_ps = psC.tile([P, H], F32, tag="linps")
                nc.tensor.matmul(
                    out=z_ps[:], lhsT=lnhT[:], rhs=w3_s[:], start=True,
                    stop=True,
                )
                # h_stage is dead after its LN2 above: reuse as z staging
                nc.vector.tensor_tensor(
                    out=h_stage[:, sl], in0=z_ps[:], in1=b3_s[:], op=ALU.add
                )
            zw = ntile_node * H
            elu_wide(h_stage[:, 0:zw], h_stage[:, 0:zw], zw)
            for t in range(ntile_node):
                sl = slice(t * H, (t + 1) * H)
                tr2_ps = psD.tile([P, P], F32, tag="trps")
                nc.tensor.transpose(
                    out=tr2_ps[:H, :], in_=h_stage[:, sl], identity=ident[:]
                )
                zT = wpool.tile([H, P], F32, tag="aggTs")
                nc.scalar.copy(zT[:], tr2_ps[:H, :])
                o_ps = psC.tile([P, H], F32, tag="linps")
                nc.tensor.matmul(
                    out=o_ps[:, :OUT], lhsT=zT[:], rhs=w4_s[:], start=True,
                    stop=True,
                )
                osl = slice(t * OUT, (t + 1) * OUT)
                nc.vector.tensor_tensor(
                    out=out_stage[:, osl], in0=o_ps[:, :OUT], in1=b4_s[:],
                    op=ALU.add,
                )
                if (t + 1) % 16 == 0:
                    c0 = t + 1 - 16
                    ov = out_d[c0 * P : (t + 1) * P, :].rearrange(
                        "(t p) f -> p t f", p=P
                    )
                    nc.scalar.dma_start(
                        out=ov,
                        in_=out_stage[
                            :, c0 * OUT : (t + 1) * OUT
                        ].rearrange("p (t f) -> p t f", f=OUT),
                    )

    nc.compile()
    return nc


def _replicate(v, parts=P):
    return np.ascontiguousarray(
        np.tile(np.asarray(v, np.float32)[None, :], (parts, 1))
    )


def kernel(
    x,
    edge_weight,
    src,
    dst,
    ln1_g,
    ln1_b,
    w1,
    b1,
    ln_g,
    ln_b,
    conv_w,
    conv_b,
    ln2_g,
    ln2_b,
    w3,
    b3,
    w4,
    b4,
    _n_cores=NC_CORES,
    _trace=False,
):
    x = np.asarray(x, np.float32)
    n_nodes = x.shape[0]
    npc = n_nodes // NC_CORES

    ln_identity = (
        np.all(ln1_g == 1) and np.all(ln1_b == 0)
        and np.all(ln_g == 1) and np.all(ln_b == 0)
        and np.all(ln2_g == 1) and np.all(ln2_b == 0)
    )

    maps, K = prep_inputs(x, np.asarray(edge_weight), np.asarray(src),
                          np.asarray(dst), n_nodes, npc)

    weights = {
        "ident": np.eye(P, dtype=np.float32),
        "ident2": np.ascontiguousarray(
            np.tile(np.eye(H, dtype=np.float32), (2, 1))
        ),
        "w1": np.asarray(w1, np.float32),
        "b1r": _replicate(b1),
        "w3": np.asarray(w3, np.float32),
        "b3r": _replicate(b3),
        "w4": np.asarray(w4, np.float32),
        "b4r": _replicate(b4),
    }
    for i in range(L):
        weights[f"cw{i}"] = np.asarray(conv_w[i], np.float32)
        weights[f"cb{i}r"] = _replicate(conv_b[i])
    if not ln_identity:
        weights["ln1g"] = _replicate(ln1_g)
        weights["ln1b"] = _replicate(ln1_b)
        weights["lng"] = _replicate(ln_g)
        weights["lnb"] = _replicate(ln_b)
        weights["ln2g"] = _replicate(ln2_g)
        weights["ln2b"] = _replicate(ln2_b)

    in_maps = [{**m, **weights} for m in maps]

    nc = build_nc(n_nodes, npc, K, ln_identity)
    res = run_bass_kernel_spmd(
        nc, in_maps, core_ids=list(range(NC_CORES)), trace=_trace
    )
    global LAST_RESULTS
    LAST_RESULTS = res
    return np.concatenate([r["out"] for r in res.results], axis=0)


LAST_RESULTS = None
